# revision 6
# baseline (speedup 1.0000x reference)
"""Masked (expander) linear layer on 8 Trainium2 NeuronCores.

Computes out = x @ (W * M)^T for
  x: [16384, 2048] f32, W: [2048, 2048] f32, M: [2048, 2048] int32 (0/1)

Sharding: pure data-parallel over rows of x. Each of the 8 cores gets 2048
rows of x plus a replicated (transposed) copy of W and M, computes its
[2048, 2048] output shard entirely locally (mask-multiply on DVE, matmul on
PE), and the host concatenates shards. No collectives.

Device-side layout choices:
 - W and M are passed transposed ([IN, OUT], contiguous) so rhs tiles
   already have the contraction dim on SBUF partitions. The mask is passed
   as int8 (values 0/1 - lossless) to cut DMA traffic.
 - x tiles are transposed on-device with PE transpose-mode matmuls
   (identity trick), PSUM -> SBUF evacuated on DVE.
 - Matmuls run in float32r mode (single-pass PE streaming, 1 cycle/row at
   free dim >= 256, vs 4 cycles/row for plain fp32). The walrus verifier
   requires every producer of an f32r matmul operand to round to f32r, so
   the DVE mask-multiply writes wm as f32r and the DVE PSUM-evacuation
   copy writes xT tiles as f32r. PSUM accumulation stays fp32 over K=2048.
"""

from contextlib import ExitStack

import numpy as np

import concourse.bass as bass
import concourse.bacc as bacc
import concourse.mybir as mybir
import concourse.tile as tile
from concourse.bass_utils import run_bass_kernel_spmd
from concourse.masks import make_identity

N_CORES = 8
P = 128

FULL_N, FULL_OUT, FULL_IN = 16384, 2048, 2048

MASK_DTYPES = {
    "int8": (mybir.dt.int8, np.int8),
    "int32": (mybir.dt.int32, np.int32),
    "float32": (mybir.dt.float32, np.float32),
}


def build_nc(
    rows: int = FULL_N // N_CORES,
    in_dim: int = FULL_IN,
    out_dim: int = FULL_OUT,
    mm_dtype=mybir.dt.float32r,
    mask_dtype: str = "int8",
    n_chunk: int = 512,
    h_chunk: int = 256,
):
    """Per-core Bass module: y[rows, out] = x[rows, in] @ (wt * m)[in, out].

    wt/mk are stored K-major in DRAM ([in_dim, out_dim], i.e. weight.T).
    """
    assert rows % P == 0 and in_dim % P == 0
    assert out_dim % n_chunk == 0 and n_chunk % h_chunk == 0
    KT = in_dim // P
    MT = rows // P
    NT = out_dim // n_chunk
    HPN = n_chunk // h_chunk  # half-panels per n-chunk

    mdt, _ = MASK_DTYPES[mask_dtype]

    nc = bacc.Bacc("TRN2", target_bir_lowering=False, debug=False)
    x = nc.dram_tensor("x", [rows, in_dim], mybir.dt.float32, kind="ExternalInput")
    wt = nc.dram_tensor("wt", [in_dim, out_dim], mybir.dt.float32, kind="ExternalInput")
    mk = nc.dram_tensor("mk", [in_dim, out_dim], mdt, kind="ExternalInput")
    y = nc.dram_tensor("y", [rows, out_dim], mybir.dt.float32, kind="ExternalOutput")

    # K-major DRAM views of the transposed weight/mask: [p, kt, n]
    wt_v = wt[:, :].rearrange("(kt p) n -> p kt n", p=P)
    mk_v = mk[:, :].rearrange("(kt p) n -> p kt n", p=P)

    with ExitStack() as ctx:
        tc = ctx.enter_context(tile.TileContext(nc))
        const_pool = ctx.enter_context(tc.tile_pool(name="const", bufs=1))
        wm_pool = ctx.enter_context(tc.tile_pool(name="wm", bufs=1))
        ws_pool = ctx.enter_context(tc.tile_pool(name="ws", bufs=1))
        msk_pool = ctx.enter_context(tc.tile_pool(name="msk", bufs=2))
        xs_pool = ctx.enter_context(tc.tile_pool(name="xs", bufs=2))
        xt_pool = ctx.enter_context(tc.tile_pool(name="xt", bufs=2))
        yo_pool = ctx.enter_context(tc.tile_pool(name="yo", bufs=3))
        pt_pool = ctx.enter_context(tc.tile_pool(name="pt", bufs=2, space="PSUM"))
        pm_pool = ctx.enter_context(tc.tile_pool(name="pm", bufs=4, space="PSUM"))

        ident = const_pool.tile([P, P], mybir.dt.float32)
        make_identity(nc, ident[:])

        # Resident masked weight, one tile per n-chunk: wm_t[nt][p, kt, n_chunk]
        wm_t = [
            wm_pool.tile([P, KT, n_chunk], mm_dtype, tag=f"wm{nt}", name=f"wm{nt}")
            for nt in range(NT)
        ]

        # ---- prep: load W/M by half-panels, mask-multiply into wm (f32r) ----
        for nt in range(NT):
            for h in range(HPN):
                n0 = nt * n_chunk + h * h_chunk
                wstage = ws_pool.tile([P, KT, h_chunk], mybir.dt.float32, tag="ws")
                nc.sync.dma_start(out=wstage[:], in_=wt_v[:, :, n0 : n0 + h_chunk])
                mtile = msk_pool.tile([P, KT, h_chunk], mdt, tag="mt")
                nc.sync.dma_start(out=mtile[:], in_=mk_v[:, :, n0 : n0 + h_chunk])
                hsl = bass.ds(h * h_chunk, h_chunk)
                for kt in range(KT):
                    nc.vector.tensor_mul(
                        wm_t[nt][:, kt, hsl], wstage[:, kt, :], mtile[:, kt, :]
                    )

        # ---- main: stream x, transpose on PE, matmul, store ----
        for mt in range(MT):
            xs = xs_pool.tile([P, in_dim], mybir.dt.float32, tag="xs")
            nc.sync.dma_start(out=xs[:], in_=x[mt * P : (mt + 1) * P, :])

            xts = []
            for kt in range(KT):
                ptile = pt_pool.tile([P, P], mybir.dt.float32, tag="pt")
                nc.tensor.transpose(ptile[:], xs[:, bass.ts(kt, P)], ident[:])
                xtile = xt_pool.tile([P, P], mm_dtype, tag=f"xt{kt}")
                nc.vector.tensor_copy(xtile[:], ptile[:])
                xts.append(xtile)

            for nt in range(NT):
                pm = pm_pool.tile([P, n_chunk], mybir.dt.float32, tag="pm")
                for kt in range(KT):
                    nc.tensor.matmul(
                        pm[:],
                        xts[kt][:],
                        wm_t[nt][:, kt, :],
                        start=(kt == 0),
                        stop=(kt == KT - 1),
                    )
                yo = yo_pool.tile([P, n_chunk], mybir.dt.float32, tag="yo")
                nc.scalar.copy(yo[:], pm[:])
                nc.sync.dma_start(
                    out=y[mt * P : (mt + 1) * P, bass.ts(nt, n_chunk)], in_=yo[:]
                )

    nc.compile()
    return nc


def _prep_host(input_, weight, mask, mask_dtype="int8"):
    _, npdt = MASK_DTYPES[mask_dtype]
    wt = np.ascontiguousarray(weight.T)
    mk = np.ascontiguousarray(mask.T).astype(npdt)
    rows = input_.shape[0] // N_CORES
    in_maps = [
        {"x": input_[c * rows : (c + 1) * rows], "wt": wt, "mk": mk}
        for c in range(N_CORES)
    ]
    return in_maps


_CACHE = {}


def _run(input_, weight, mask, trace=False, **build_kw):
    rows_total, in_dim = input_.shape
    out_dim = weight.shape[0]
    key = (rows_total, in_dim, out_dim, tuple(sorted(build_kw.items())))
    if key not in _CACHE:
        _CACHE[key] = build_nc(
            rows=rows_total // N_CORES, in_dim=in_dim, out_dim=out_dim, **build_kw
        )
    nc = _CACHE[key]
    in_maps = _prep_host(input_, weight, mask, build_kw.get("mask_dtype", "int8"))
    res = run_bass_kernel_spmd(nc, in_maps, core_ids=list(range(N_CORES)), trace=trace)
    out = np.concatenate([res.results[c]["y"] for c in range(N_CORES)], axis=0)
    return out, res


def kernel(input_, weight, mask):
    input_ = np.asarray(input_, dtype=np.float32)
    weight = np.asarray(weight, dtype=np.float32)
    mask = np.asarray(mask)
    out, _ = _run(input_, weight, mask, trace=False)
    return out


# revision 7
# speedup vs baseline: 1.0900x; 1.0900x over previous
"""Masked (expander) linear layer on 8 Trainium2 NeuronCores.

Computes out = x @ (W * M)^T for
  x: [16384, 2048] f32, W: [2048, 2048] f32, M: [2048, 2048] int32 (0/1)

Sharding: pure data-parallel over rows of x. Each of the 8 cores gets 2048
rows of x plus a replicated (transposed) copy of W and M, computes its
[2048, 2048] output shard entirely locally (mask-multiply on DVE, matmul on
PE), and the host concatenates shards. No collectives.

Device-side design:
 - W and M are passed transposed and panel-major ([NT, IN, n_chunk],
   contiguous per panel) so the contraction dim lands on SBUF partitions
   and each panel loads as one large near-contiguous DMA. The mask is
   passed as int8 (values 0/1 - lossless repack) to cut DMA traffic.
 - x tiles are transposed on-device with PE transpose-mode matmuls
   (identity trick); PSUM -> SBUF evacuation on DVE writes f32r.
 - Matmuls run in float32r mode (single-pass PE streaming, 1 cycle/row at
   free dim >= 256, vs 4 cycles/row for plain fp32; ~1.3e-4 rel err at
   K=2048). The walrus verifier requires f32r matmul operands to be
   produced by f32r-rounding instructions: the DVE mask-multiply writes
   wm as f32r, the DVE PSUM-evacuation copy writes xT as f32r.
 - m-tiles are processed in blocks of 4 with the n-chunk loop outside the
   in-block m loop, so the first weight panel's matmul work (~4x16 MMs)
   covers the DMA time of later panels - keeps PE gapless during the
   weight-load head and HAM at full clock.
"""

from contextlib import ExitStack

import numpy as np

import concourse.bacc as bacc
import concourse.bass as bass
import concourse.mybir as mybir
import concourse.tile as tile
from concourse.bass_utils import run_bass_kernel_spmd
from concourse.masks import make_identity

N_CORES = 8
P = 128

FULL_N, FULL_OUT, FULL_IN = 16384, 2048, 2048

MASK_DTYPES = {
    "int8": (mybir.dt.int8, np.int8),
    "int32": (mybir.dt.int32, np.int32),
    "float32": (mybir.dt.float32, np.float32),
}


def build_nc(
    rows: int = FULL_N // N_CORES,
    in_dim: int = FULL_IN,
    out_dim: int = FULL_OUT,
    mm_dtype=mybir.dt.float32r,
    mask_dtype: str = "int8",
    n_chunk: int = 512,
    m_block: int = 4,
):
    """Per-core Bass module: y[rows, out] = x[rows, in] @ (wt * m)[in, out].

    wt/mk are stored panel-major in DRAM: [NT, in_dim, n_chunk] (weight.T
    split into NT contiguous column panels).
    """
    assert rows % P == 0 and in_dim % P == 0 and out_dim % n_chunk == 0
    KT = in_dim // P
    MT = rows // P
    NT = out_dim // n_chunk
    KH = KT // 2  # k-half used for W staging granularity
    assert MT % m_block == 0

    mdt, _ = MASK_DTYPES[mask_dtype]

    nc = bacc.Bacc("TRN2", target_bir_lowering=False, debug=False)
    x = nc.dram_tensor("x", [rows, in_dim], mybir.dt.float32, kind="ExternalInput")
    wt = nc.dram_tensor(
        "wt", [NT, in_dim, n_chunk], mybir.dt.float32, kind="ExternalInput"
    )
    mk = nc.dram_tensor("mk", [NT, in_dim, n_chunk], mdt, kind="ExternalInput")
    y = nc.dram_tensor("y", [rows, out_dim], mybir.dt.float32, kind="ExternalOutput")

    # K-major DRAM views per panel: [p, kt, n]
    wt_v = wt[:, :, :].rearrange("t (kt p) n -> t p kt n", p=P)
    mk_v = mk[:, :, :].rearrange("t (kt p) n -> t p kt n", p=P)

    with ExitStack() as ctx:
        tc = ctx.enter_context(tile.TileContext(nc))
        const_pool = ctx.enter_context(tc.tile_pool(name="const", bufs=1))
        wm_pool = ctx.enter_context(tc.tile_pool(name="wm", bufs=1))
        ws_pool = ctx.enter_context(tc.tile_pool(name="ws", bufs=1))
        msk_pool = ctx.enter_context(tc.tile_pool(name="msk", bufs=2))
        xs_pool = ctx.enter_context(tc.tile_pool(name="xs", bufs=2))
        xt_pool = ctx.enter_context(tc.tile_pool(name="xt", bufs=1))
        yo_pool = ctx.enter_context(tc.tile_pool(name="yo", bufs=3))
        pt_pool = ctx.enter_context(tc.tile_pool(name="pt", bufs=2, space="PSUM"))
        pm_pool = ctx.enter_context(tc.tile_pool(name="pm", bufs=4, space="PSUM"))

        ident = const_pool.tile([P, P], mybir.dt.float32)
        make_identity(nc, ident[:])

        # Resident masked weight, one tile per n-chunk: wm_t[nt][p, kt, n_chunk]
        wm_t = [
            wm_pool.tile([P, KT, n_chunk], mm_dtype, tag=f"wm{nt}", name=f"wm{nt}")
            for nt in range(NT)
        ]

        # ---- prep: load W/M per panel in k-halves, mask-multiply into wm ----
        for nt in range(NT):
            for h in range(2):
                ksl = slice(h * KH, (h + 1) * KH)
                wstage = ws_pool.tile([P, KH, n_chunk], mybir.dt.float32, tag="ws")
                nc.sync.dma_start(out=wstage[:], in_=wt_v[nt, :, ksl, :])
                mtile = msk_pool.tile([P, KH, n_chunk], mdt, tag="mt")
                nc.sync.dma_start(out=mtile[:], in_=mk_v[nt, :, ksl, :])
                for k in range(KH):
                    nc.vector.tensor_mul(
                        wm_t[nt][:, h * KH + k, :], wstage[:, k, :], mtile[:, k, :]
                    )

        # ---- main: blocks of m_block m-tiles; nt-outer inside a block ----
        for mb0 in range(0, MT, m_block):
            xts = {}
            for mb in range(m_block):
                mt = mb0 + mb
                xs = xs_pool.tile([P, in_dim], mybir.dt.float32, tag="xs")
                nc.sync.dma_start(out=xs[:], in_=x[mt * P : (mt + 1) * P, :])
                for kt in range(KT):
                    ptile = pt_pool.tile([P, P], mybir.dt.float32, tag="pt")
                    nc.tensor.transpose(ptile[:], xs[:, bass.ts(kt, P)], ident[:])
                    xtile = xt_pool.tile(
                        [P, P], mm_dtype, tag=f"xt{mb}_{kt}", name=f"xt{mb}_{kt}"
                    )
                    nc.vector.tensor_copy(xtile[:], ptile[:])
                    xts[(mb, kt)] = xtile

            for nt in range(NT):
                for mb in range(m_block):
                    mt = mb0 + mb
                    pm = pm_pool.tile([P, n_chunk], mybir.dt.float32, tag="pm")
                    for kt in range(KT):
                        nc.tensor.matmul(
                            pm[:],
                            xts[(mb, kt)][:],
                            wm_t[nt][:, kt, :],
                            start=(kt == 0),
                            stop=(kt == KT - 1),
                        )
                    yo = yo_pool.tile([P, n_chunk], mybir.dt.float32, tag="yo")
                    nc.scalar.copy(yo[:], pm[:])
                    nc.sync.dma_start(
                        out=y[mt * P : (mt + 1) * P, bass.ts(nt, n_chunk)], in_=yo[:]
                    )

    nc.compile()
    return nc


def _prep_host(input_, weight, mask, mask_dtype="int8", n_chunk=512):
    _, npdt = MASK_DTYPES[mask_dtype]
    in_dim, out_dim = weight.shape[1], weight.shape[0]
    nt = out_dim // n_chunk
    # weight.T -> [NT, IN, n_chunk], each panel contiguous
    wtp = np.ascontiguousarray(
        weight.T.reshape(in_dim, nt, n_chunk).transpose(1, 0, 2)
    )
    mkp = np.ascontiguousarray(
        mask.T.reshape(in_dim, nt, n_chunk).transpose(1, 0, 2)
    ).astype(npdt)
    rows = input_.shape[0] // N_CORES
    in_maps = [
        {"x": input_[c * rows : (c + 1) * rows], "wt": wtp, "mk": mkp}
        for c in range(N_CORES)
    ]
    return in_maps


_CACHE = {}


def _run(input_, weight, mask, trace=False, **build_kw):
    rows_total, in_dim = input_.shape
    out_dim = weight.shape[0]
    key = (rows_total, in_dim, out_dim, tuple(sorted(build_kw.items())))
    if key not in _CACHE:
        _CACHE[key] = build_nc(
            rows=rows_total // N_CORES, in_dim=in_dim, out_dim=out_dim, **build_kw
        )
    nc = _CACHE[key]
    in_maps = _prep_host(
        input_,
        weight,
        mask,
        build_kw.get("mask_dtype", "int8"),
        build_kw.get("n_chunk", 512),
    )
    res = run_bass_kernel_spmd(nc, in_maps, core_ids=list(range(N_CORES)), trace=trace)
    out = np.concatenate([res.results[c]["y"] for c in range(N_CORES)], axis=0)
    return out, res


def kernel(input_, weight, mask):
    input_ = np.asarray(input_, dtype=np.float32)
    weight = np.asarray(weight, dtype=np.float32)
    mask = np.asarray(mask)
    out, _ = _run(input_, weight, mask, trace=False)
    return out
